# revision 6
# baseline (speedup 1.0000x reference)
"""Block-diagonal linear for Trainium2 (8 NeuronCores, batch-data-parallel).

y[b,c,o] = sum_i x[b,c,i]*W[c,o,i] + bias[c,o], x [16384, 3072] f32.

Strategy: host pre-casts to fp16 and lays each core's shard out transposed
and group-blocked: xt[grp, p, bb, b] where block blk = 4*grp+bb covers 32
components, partition p<96 holds f-row blk*96+p (f = 3c+i), and p=96 is a
baked-in ones row. Per block the block-diagonal linear is a real TensorE
matmul: stationary lhsT [97, 96] holds 32 3x3 blocks on the diagonal plus
a bias row, moving rhs is the block's [97, 512] column slices, so PSUM
accumulates y^T[3c+o, b] in f32 with bias included. ScalarE and DVE
alternate draining PSUM -> fp16 SBUF.

DMA: one HWDGE transfer per 4-block group (1.6MB, 16KB contiguous per
partition both in DRAM and SBUF) -> near-peak HBM bandwidth; fp16 I/O
halves traffic vs f32 (cast host-side; device math is f32 PSUM accum).
~25MB HBM/core -> ~72us roofline.
"""

import numpy as np

import concourse.bacc as bacc
import concourse.mybir as mybir
from concourse import bass_utils
from concourse.tile import TileContext

N_CORES = 8
B_FULL = 16384
F = 3072
C = F // 3  # 1024
B_CORE = B_FULL // N_CORES  # 2048
CPB = 32  # components per block
KB = 3 * CPB  # 96 f-rows per block
NBLK = C // CPB  # 32 blocks
GB = 4  # blocks per DMA group
NGRP = NBLK // GB  # 8 groups
MM_N = 512  # max moving free dim
FP32 = mybir.dt.float32
FP16 = mybir.dt.float16


def build_bass():
    nc = bacc.Bacc("TRN2", num_devices=N_CORES)
    xt = nc.dram_tensor("xt", [NGRP * (KB + 1), GB * B_CORE], FP16, kind="ExternalInput")
    wst = nc.dram_tensor("wst", [KB + 1, NBLK * KB], FP16, kind="ExternalInput")
    yt = nc.dram_tensor("yt", [NGRP * KB, GB * B_CORE], FP16, kind="ExternalOutput")

    with TileContext(nc) as tc:
        with (
            tc.tile_pool(name="wpool", bufs=1) as wpool,
            tc.tile_pool(name="xpool", bufs=3) as xpool,
            tc.tile_pool(name="ypool", bufs=3) as ypool,
            tc.tile_pool(name="psum", bufs=2, space="PSUM") as psum_pool,
        ):
            w_sb = wpool.tile([KB + 1, NBLK * KB], FP16)
            nc.sync.dma_start(out=w_sb[:, :], in_=wst.ap()[:, :])

            for grp in range(NGRP):
                xg = xpool.tile([KB + 1, GB * B_CORE], FP16, tag="x", name=f"x_{grp}")
                nc.sync.dma_start(
                    out=xg[:, :],
                    in_=xt.ap()[grp * (KB + 1) : (grp + 1) * (KB + 1), :],
                )
                yg = ypool.tile([KB, GB * B_CORE], FP16, tag="y", name=f"y_{grp}")
                x3 = xg[:, :].rearrange("p (g b) -> p g b", g=GB)
                y3 = yg[:, :].rearrange("p (g b) -> p g b", g=GB)

                for bb in range(GB):
                    blk = grp * GB + bb
                    pt = psum_pool.tile([KB, B_CORE], FP32, tag="ps", name=f"ps_{blk}")
                    lhsT = w_sb[:, blk * KB : (blk + 1) * KB]
                    for j in range(B_CORE // MM_N):
                        nc.tensor.matmul(
                            out=pt[:, j * MM_N : (j + 1) * MM_N],
                            lhsT=lhsT,
                            rhs=x3[:, bb, j * MM_N : (j + 1) * MM_N],
                            start=True,
                            stop=True,
                        )
                    if blk % 2 == 0:
                        nc.scalar.copy(y3[:, bb, :], pt[:, :])
                    else:
                        nc.vector.tensor_copy(out=y3[:, bb, :], in_=pt[:, :])

                nc.sync.dma_start(
                    out=yt.ap()[grp * KB : (grp + 1) * KB, :],
                    in_=yg[:, :],
                )

    nc.compile()
    return nc


def _prep_weights(W, b):
    # wst[3*cc+i, KB*blk + 3*cc+o] = W[CPB*blk+cc, o, i]; row KB = bias
    W = np.asarray(W, np.float32)
    b = np.asarray(b, np.float32)
    wst = np.zeros((KB + 1, NBLK * KB), np.float16)
    blk_g, cc_g, o_g, i_g = np.meshgrid(
        np.arange(NBLK), np.arange(CPB), np.arange(3), np.arange(3), indexing="ij"
    )
    wst[3 * cc_g + i_g, KB * blk_g + 3 * cc_g + o_g] = W[
        CPB * blk_g + cc_g, o_g, i_g
    ].astype(np.float16)
    blk_g, cc_g, o_g = np.meshgrid(
        np.arange(NBLK), np.arange(CPB), np.arange(3), indexing="ij"
    )
    wst[KB, KB * blk_g + 3 * cc_g + o_g] = b[CPB * blk_g + cc_g, o_g].astype(
        np.float16
    )
    return wst


def _prep_x(xk):
    # xt[grp, p<96, bb, b] = xT[(4*grp+bb)*96 + p, b]; xt[grp, 96, :, :] = 1
    xT = xk.astype(np.float16).T  # [3072, 2048]
    xt = np.ones((NGRP, KB + 1, GB, B_CORE), np.float16)
    xt[:, :KB] = xT.reshape(NGRP, GB, KB, B_CORE).transpose(0, 2, 1, 3)
    return np.ascontiguousarray(xt).reshape(NGRP * (KB + 1), GB * B_CORE)


def _unprep_y(ytk):
    # yT[(4*grp+bb)*96 + p, b] = yt[grp, p, bb, b]
    yT = (
        ytk.reshape(NGRP, KB, GB, B_CORE)
        .transpose(0, 2, 1, 3)
        .reshape(F, B_CORE)
    )
    return yT.T  # [2048, 3072]


def run(x, W, b, trace=False, **run_kwargs):
    nc = build_bass()
    wst = _prep_weights(W, b)
    x = np.asarray(x, dtype=np.float32)
    in_maps = [
        {"xt": _prep_x(x[k * B_CORE : (k + 1) * B_CORE]), "wst": wst}
        for k in range(N_CORES)
    ]
    res = bass_utils.run_bass_kernel_spmd(
        nc, in_maps, core_ids=list(range(N_CORES)), trace=trace, **run_kwargs
    )
    y = np.concatenate([_unprep_y(r["yt"]) for r in res.results], axis=0).astype(
        np.float32
    )
    return y, res


def kernel(x, W, b):
    y, _ = run(x, W, b, trace=False)
    return y


# revision 7
# speedup vs baseline: 1.0015x; 1.0015x over previous
"""Block-diagonal linear for Trainium2 (8 NeuronCores, batch-data-parallel).

y[b,c,o] = sum_i x[b,c,i]*W[c,o,i] + bias[c,o], x [16384, 3072] f32.

Strategy: host pre-casts to fp16 and lays each core's shard out transposed
per 32-component block: xt[blk, p, b] where partition p<96 holds f-row
blk*96+p (f = 3c+i) and p=96 is a baked-in ones row. Per block the
block-diagonal linear is a real TensorE matmul: stationary lhsT [97, 96]
holds 32 3x3 blocks on the diagonal plus a bias row, moving rhs is the
block's [97, 512] column slices, so PSUM accumulates y^T[3c+o, b] in f32
with bias included. ScalarE and DVE alternate draining PSUM -> fp16 SBUF.

DMA geometry: per-block [97, 2048] transfers (4KB/partition runs) spray
across all 16 SDMA engines; input loads alternate between the two HWDGE
rings (SP/ACT) and output stores go via gpsimd SWDGE so loads and stores
proceed concurrently instead of serializing in one DGE FIFO. fp16 I/O
halves HBM traffic vs f32 (cast host-side; device math is f32 PSUM
accum). ~25MB HBM/core -> ~72us roofline.
"""

import numpy as np

import concourse.bacc as bacc
import concourse.mybir as mybir
from concourse import bass_utils
from concourse.tile import TileContext

N_CORES = 8
B_FULL = 16384
F = 3072
C = F // 3  # 1024
B_CORE = B_FULL // N_CORES  # 2048
CPB = 32  # components per block
KB = 3 * CPB  # 96 f-rows per block
NBLK = C // CPB  # 32 blocks
MM_N = 512  # max moving free dim
FP32 = mybir.dt.float32
FP16 = mybir.dt.float16


def build_bass():
    nc = bacc.Bacc("TRN2", num_devices=N_CORES)
    xt = nc.dram_tensor("xt", [NBLK * (KB + 1), B_CORE], FP16, kind="ExternalInput")
    wst = nc.dram_tensor("wst", [KB + 1, NBLK * KB], FP16, kind="ExternalInput")
    yt = nc.dram_tensor("yt", [NBLK * KB, B_CORE], FP16, kind="ExternalOutput")

    with TileContext(nc) as tc:
        with (
            tc.tile_pool(name="wpool", bufs=1) as wpool,
            tc.tile_pool(name="xpool", bufs=4) as xpool,
            tc.tile_pool(name="ypool", bufs=4) as ypool,
            tc.tile_pool(name="psum", bufs=2, space="PSUM") as psum_pool,
        ):
            w_sb = wpool.tile([KB + 1, NBLK * KB], FP16)
            nc.sync.dma_start(out=w_sb[:, :], in_=wst.ap()[:, :])

            for blk in range(NBLK):
                xa = xpool.tile([KB + 1, B_CORE], FP16, tag="x", name=f"x_{blk}")
                ldeng = nc.sync if blk % 2 == 0 else nc.scalar
                ldeng.dma_start(
                    out=xa[:, :],
                    in_=xt.ap()[blk * (KB + 1) : (blk + 1) * (KB + 1), :],
                )

                pt = psum_pool.tile([KB, B_CORE], FP32, tag="ps", name=f"ps_{blk}")
                lhsT = w_sb[:, blk * KB : (blk + 1) * KB]
                for j in range(B_CORE // MM_N):
                    nc.tensor.matmul(
                        out=pt[:, j * MM_N : (j + 1) * MM_N],
                        lhsT=lhsT,
                        rhs=xa[:, j * MM_N : (j + 1) * MM_N],
                        start=True,
                        stop=True,
                    )

                yb = ypool.tile([KB, B_CORE], FP16, tag="y", name=f"y_{blk}")
                if blk % 2 == 0:
                    nc.scalar.copy(yb[:, :], pt[:, :])
                else:
                    nc.vector.tensor_copy(out=yb[:, :], in_=pt[:, :])
                nc.gpsimd.dma_start(
                    out=yt.ap()[blk * KB : (blk + 1) * KB, :],
                    in_=yb[:, :],
                )

    nc.compile()
    return nc


def _prep_weights(W, b):
    # wst[3*cc+i, KB*blk + 3*cc+o] = W[CPB*blk+cc, o, i]; row KB = bias
    W = np.asarray(W, np.float32)
    b = np.asarray(b, np.float32)
    wst = np.zeros((KB + 1, NBLK * KB), np.float16)
    blk_g, cc_g, o_g, i_g = np.meshgrid(
        np.arange(NBLK), np.arange(CPB), np.arange(3), np.arange(3), indexing="ij"
    )
    wst[3 * cc_g + i_g, KB * blk_g + 3 * cc_g + o_g] = W[
        CPB * blk_g + cc_g, o_g, i_g
    ].astype(np.float16)
    blk_g, cc_g, o_g = np.meshgrid(
        np.arange(NBLK), np.arange(CPB), np.arange(3), indexing="ij"
    )
    wst[KB, KB * blk_g + 3 * cc_g + o_g] = b[CPB * blk_g + cc_g, o_g].astype(
        np.float16
    )
    return wst


def _prep_x(xk):
    # xt[blk, p<96, b] = xT[blk*96 + p, b]; xt[blk, 96, :] = 1
    xT = xk.astype(np.float16).T  # [3072, 2048]
    xt = np.ones((NBLK, KB + 1, B_CORE), np.float16)
    xt[:, :KB] = xT.reshape(NBLK, KB, B_CORE)
    return np.ascontiguousarray(xt).reshape(NBLK * (KB + 1), B_CORE)


def _unprep_y(ytk):
    # yT[blk*96 + p, b] = yt[blk, p, b]
    return ytk.T  # [2048, 3072]


def run(x, W, b, trace=False, **run_kwargs):
    nc = build_bass()
    wst = _prep_weights(W, b)
    x = np.asarray(x, dtype=np.float32)
    in_maps = [
        {"xt": _prep_x(x[k * B_CORE : (k + 1) * B_CORE]), "wst": wst}
        for k in range(N_CORES)
    ]
    res = bass_utils.run_bass_kernel_spmd(
        nc, in_maps, core_ids=list(range(N_CORES)), trace=trace, **run_kwargs
    )
    y = np.concatenate([_unprep_y(r["yt"]) for r in res.results], axis=0).astype(
        np.float32
    )
    return y, res


def kernel(x, W, b):
    y, _ = run(x, W, b, trace=False)
    return y


# revision 8
# speedup vs baseline: 5.6186x; 5.6100x over previous
"""Block-diagonal linear for Trainium2 (8 NeuronCores, batch-data-parallel).

y[b,c,o] = sum_i x[b,c,i]*W[c,o,i] + bias[c,o], x [16384, 3072] f32.

Strategy: host pre-casts to fp16 and transposes each core's shard to
xT [3072, 2048] (f-major). Per 32-component block the block-diagonal
linear is a real TensorE matmul: stationary lhsT [96, 96] holds 32 3x3
blocks on the diagonal, moving rhs is the block's [96, 512] column
slices, PSUM accumulates y^T[3c+o, b] in f32. In this layout bias[c,o]
is a per-partition scalar, so the PSUM->SBUF drain folds it in for free:
ScalarE activation-Identity(+bias AP) and DVE tensor_scalar(+bias AP)
alternate per block.

DMA geometry notes (hard-won): HWDGE splits a transfer across the 16
SDMA engines only when the partition count divides evenly — [97, N]
transfers collapse onto ONE engine (~26 GB/s), [96, N] spray. Loads
alternate between the two HWDGE rings (SP/ACT), stores go via gpsimd
SWDGE, so loads and stores proceed concurrently instead of serializing
in one DGE FIFO. fp16 I/O halves HBM traffic vs f32 (cast host-side;
accumulation is f32 in PSUM). ~25MB HBM/core -> ~71us roofline.
"""

import numpy as np

import concourse.bacc as bacc
import concourse.mybir as mybir
from concourse import bass_utils
from concourse.tile import TileContext

N_CORES = 8
B_FULL = 16384
F = 3072
C = F // 3  # 1024
B_CORE = B_FULL // N_CORES  # 2048
CPB = 32  # components per block
KB = 3 * CPB  # 96 f-rows per block
NBLK = C // CPB  # 32 blocks
MM_N = 512  # max moving free dim
FP32 = mybir.dt.float32
FP16 = mybir.dt.float16
ADD = mybir.AluOpType.add


def build_bass():
    nc = bacc.Bacc("TRN2", num_devices=N_CORES)
    xt = nc.dram_tensor("xt", [F, B_CORE], FP16, kind="ExternalInput")
    wst = nc.dram_tensor("wst", [KB, NBLK * KB], FP16, kind="ExternalInput")
    bst = nc.dram_tensor("bst", [KB, NBLK], FP32, kind="ExternalInput")
    yt = nc.dram_tensor("yt", [F, B_CORE], FP16, kind="ExternalOutput")

    with TileContext(nc) as tc:
        with (
            tc.tile_pool(name="wpool", bufs=1) as wpool,
            tc.tile_pool(name="xpool", bufs=4) as xpool,
            tc.tile_pool(name="ypool", bufs=4) as ypool,
            tc.tile_pool(name="psum", bufs=2, space="PSUM") as psum_pool,
        ):
            w_sb = wpool.tile([KB, NBLK * KB], FP16)
            nc.sync.dma_start(out=w_sb[:, :], in_=wst.ap()[:, :])
            b_sb = wpool.tile([KB, NBLK], FP32)
            nc.sync.dma_start(out=b_sb[:, :], in_=bst.ap()[:, :])

            for blk in range(NBLK):
                xa = xpool.tile([KB, B_CORE], FP16, tag="x", name=f"x_{blk}")
                ldeng = nc.sync if blk % 2 == 0 else nc.scalar
                ldeng.dma_start(
                    out=xa[:, :],
                    in_=xt.ap()[blk * KB : (blk + 1) * KB, :],
                )

                pt = psum_pool.tile([KB, B_CORE], FP32, tag="ps", name=f"ps_{blk}")
                lhsT = w_sb[:, blk * KB : (blk + 1) * KB]
                for j in range(B_CORE // MM_N):
                    nc.tensor.matmul(
                        out=pt[:, j * MM_N : (j + 1) * MM_N],
                        lhsT=lhsT,
                        rhs=xa[:, j * MM_N : (j + 1) * MM_N],
                        start=True,
                        stop=True,
                    )

                yb = ypool.tile([KB, B_CORE], FP16, tag="y", name=f"y_{blk}")
                bias_col = b_sb[:, blk : blk + 1]
                if blk % 2 == 0:
                    nc.scalar.add(yb[:, :], pt[:, :], bias_col)
                else:
                    nc.vector.tensor_scalar(
                        out=yb[:, :],
                        in0=pt[:, :],
                        scalar1=bias_col,
                        scalar2=None,
                        op0=ADD,
                    )
                nc.gpsimd.dma_start(
                    out=yt.ap()[blk * KB : (blk + 1) * KB, :],
                    in_=yb[:, :],
                )

    nc.compile()
    return nc


def _prep_weights(W, b):
    # wst[3*cc+i, KB*blk + 3*cc+o] = W[CPB*blk+cc, o, i]
    W = np.asarray(W, np.float32)
    b = np.asarray(b, np.float32)
    wst = np.zeros((KB, NBLK * KB), np.float16)
    blk_g, cc_g, o_g, i_g = np.meshgrid(
        np.arange(NBLK), np.arange(CPB), np.arange(3), np.arange(3), indexing="ij"
    )
    wst[3 * cc_g + i_g, KB * blk_g + 3 * cc_g + o_g] = W[
        CPB * blk_g + cc_g, o_g, i_g
    ].astype(np.float16)
    # bst[3*cc+o, blk] = b[CPB*blk+cc, o]
    bst = np.ascontiguousarray(
        b.reshape(NBLK, CPB * 3).T
    )  # [96, 32] f32
    return wst, bst


def run(x, W, b, trace=False, **run_kwargs):
    nc = build_bass()
    wst, bst = _prep_weights(W, b)
    x = np.asarray(x, dtype=np.float32)
    in_maps = [
        {
            "xt": np.ascontiguousarray(
                x[k * B_CORE : (k + 1) * B_CORE].astype(np.float16).T
            ),
            "wst": wst,
            "bst": bst,
        }
        for k in range(N_CORES)
    ]
    res = bass_utils.run_bass_kernel_spmd(
        nc, in_maps, core_ids=list(range(N_CORES)), trace=trace, **run_kwargs
    )
    y = np.concatenate([r["yt"].T for r in res.results], axis=0).astype(np.float32)
    return y, res


def kernel(x, W, b):
    y, _ = run(x, W, b, trace=False)
    return y


# revision 11
# speedup vs baseline: 5.7340x; 1.0205x over previous
"""Block-diagonal linear for Trainium2 (8 NeuronCores, batch-data-parallel).

y[b,c,o] = sum_i x[b,c,i]*W[c,o,i] + bias[c,o], x [16384, 3072] f32.

Strategy: host pre-casts to fp16 and transposes each core's shard to
xT [3072, 2048] (f-major). Per 32-component block the block-diagonal
linear is a real TensorE matmul: stationary lhsT [96, 96] holds 32 3x3
blocks on the diagonal, moving rhs is the block's [96, 512] column
slices, PSUM accumulates y^T[3c+o, b] in f32. In this layout bias[c,o]
is a per-partition scalar, so the PSUM->SBUF drain folds it in for free:
ScalarE activation-Identity(+bias AP) and DVE tensor_scalar(+bias AP)
alternate per block.

DMA geometry notes (hard-won): HWDGE splits a transfer across the 16
SDMA engines only when the partition count divides evenly — [97, N]
transfers collapse onto ONE engine (~26 GB/s), [96, N] spray. Loads
alternate between the two HWDGE rings (SP/ACT), stores go via gpsimd
SWDGE, so loads and stores proceed concurrently instead of serializing
in one DGE FIFO. fp16 I/O halves HBM traffic vs f32 (cast host-side;
accumulation is f32 in PSUM). ~25MB HBM/core -> ~71us roofline.
"""

import numpy as np

import concourse.bacc as bacc
import concourse.mybir as mybir
from concourse import bass_utils
from concourse.tile import TileContext

N_CORES = 8
B_FULL = 16384
F = 3072
C = F // 3  # 1024
B_CORE = B_FULL // N_CORES  # 2048
CPB = 32  # components per block
KB = 3 * CPB  # 96 f-rows per block
NBLK = C // CPB  # 32 blocks
MM_N = 512  # max moving free dim
FP32 = mybir.dt.float32
FP16 = mybir.dt.float16
ADD = mybir.AluOpType.add


def build_bass():
    nc = bacc.Bacc("TRN2", num_devices=N_CORES)
    xt = nc.dram_tensor("xt", [F, B_CORE], FP16, kind="ExternalInput")
    # stationary padded to 128 columns so FWL (fast weight load) kicks in
    wst = nc.dram_tensor("wst", [KB, NBLK * 128], FP16, kind="ExternalInput")
    bst = nc.dram_tensor("bst", [KB, NBLK], FP32, kind="ExternalInput")
    yt = nc.dram_tensor("yt", [F, B_CORE], FP16, kind="ExternalOutput")

    with TileContext(nc) as tc:
        with (
            tc.tile_pool(name="wpool", bufs=1) as wpool,
            tc.tile_pool(name="xpool", bufs=10) as xpool,
            tc.tile_pool(name="ypool", bufs=6) as ypool,
            tc.tile_pool(name="psum", bufs=2, space="PSUM") as psum_pool,
        ):
            w_sb = wpool.tile([KB, NBLK * 128], FP16)
            nc.gpsimd.dma_start(out=w_sb[:, :], in_=wst.ap()[:, :])
            b_sb = wpool.tile([KB, NBLK], FP32)
            nc.gpsimd.dma_start(out=b_sb[:, :], in_=bst.ap()[:, :])

            for blk in range(NBLK):
                xa = xpool.tile([KB, B_CORE], FP16, tag="x", name=f"x_{blk}")
                ldeng = nc.sync if blk % 2 == 0 else nc.scalar
                ldeng.dma_start(
                    out=xa[:, :],
                    in_=xt.ap()[blk * KB : (blk + 1) * KB, :],
                )

                pt = psum_pool.tile([128, B_CORE], FP32, tag="ps", name=f"ps_{blk}")
                lhsT = w_sb[:, blk * 128 : (blk + 1) * 128]
                for j in range(B_CORE // MM_N):
                    nc.tensor.matmul(
                        out=pt[:, j * MM_N : (j + 1) * MM_N],
                        lhsT=lhsT,
                        rhs=xa[:, j * MM_N : (j + 1) * MM_N],
                        start=True,
                        stop=True,
                    )

                yb = ypool.tile([KB, B_CORE], FP16, tag="y", name=f"y_{blk}")
                bias_col = b_sb[:, blk : blk + 1]
                if blk % 2 == 0:
                    nc.scalar.add(yb[:, :], pt[:KB, :], bias_col)
                else:
                    nc.vector.tensor_scalar(
                        out=yb[:, :],
                        in0=pt[:KB, :],
                        scalar1=bias_col,
                        scalar2=None,
                        op0=ADD,
                    )
                nc.gpsimd.dma_start(
                    out=yt.ap()[blk * KB : (blk + 1) * KB, :],
                    in_=yb[:, :],
                )

    nc.compile()
    return nc


def _prep_weights(W, b):
    # wst[3*cc+i, 128*blk + 3*cc+o] = W[CPB*blk+cc, o, i]; cols 96-127 zero pad
    W = np.asarray(W, np.float32)
    b = np.asarray(b, np.float32)
    wst = np.zeros((KB, NBLK * 128), np.float16)
    blk_g, cc_g, o_g, i_g = np.meshgrid(
        np.arange(NBLK), np.arange(CPB), np.arange(3), np.arange(3), indexing="ij"
    )
    wst[3 * cc_g + i_g, 128 * blk_g + 3 * cc_g + o_g] = W[
        CPB * blk_g + cc_g, o_g, i_g
    ].astype(np.float16)
    # bst[3*cc+o, blk] = b[CPB*blk+cc, o]
    bst = np.ascontiguousarray(
        b.reshape(NBLK, CPB * 3).T
    )  # [96, 32] f32
    return wst, bst


def run(x, W, b, trace=False, **run_kwargs):
    nc = build_bass()
    wst, bst = _prep_weights(W, b)
    x = np.asarray(x, dtype=np.float32)
    in_maps = [
        {
            "xt": np.ascontiguousarray(
                x[k * B_CORE : (k + 1) * B_CORE].astype(np.float16).T
            ),
            "wst": wst,
            "bst": bst,
        }
        for k in range(N_CORES)
    ]
    res = bass_utils.run_bass_kernel_spmd(
        nc, in_maps, core_ids=list(range(N_CORES)), trace=trace, **run_kwargs
    )
    y = np.concatenate([r["yt"].T for r in res.results], axis=0).astype(np.float32)
    return y, res


def kernel(x, W, b):
    y, _ = run(x, W, b, trace=False)
    return y
